# revision 14
# baseline (speedup 1.0000x reference)
"""Trainium2 Bass kernel for nn_ConnectionsSMP (segment-max pooling + pairwise relation MLP).

Contract: kernel(**inputs) takes FULL numpy inputs
  encoded [8, 64, 512, 512] f32, masks [8, 1, 512, 512] i32,
  w1 [128, 32] f32, b1 [32] f32, w2 [32, 4] f32, b2 [4] f32
and returns (vectors [8, 32, 64] f32, connections [8, 4, 32, 32] f32),
matching the reference:
  vectors[b, k, f]  = max(0, segment_max(encoded[b, f], masks[b] == k+1))
  connections[b, c, j, i] = sigmoid(concat(v_i, v_j) @ w1 @ w2 + b1 @ w2 + b2)[c]

Sharding: data-parallel over batch B=8 across the 8 NeuronCores; the small
MLP weights are replicated (the two linear layers are folded on-device:
out = sigmoid(v_i @ A + v_j @ B + c0) with [A; B] = w1 @ w2, c0 = b1 @ w2 + b2).
"""

import numpy as np

B, F, H, W = 8, 64, 512, 512
HW = H * W
NSEG = 32          # instance ids 1..32 (id 0 = background, dropped)
F2 = 32            # hidden dim of MLP
C = 4              # output channels

_cached = {}


def _build_program():
    import concourse.bass as bass
    import concourse.tile as tile
    from concourse import bacc, mybir

    f32 = mybir.dt.float32
    i32 = mybir.dt.int32
    Alu = mybir.AluOpType
    Act = mybir.ActivationFunctionType

    nc = bacc.Bacc()
    enc = nc.dram_tensor("encoded", [F, HW], f32, kind="ExternalInput")
    msk = nc.dram_tensor("masks", [HW], i32, kind="ExternalInput")
    w1 = nc.dram_tensor("w1", [2 * F, F2], f32, kind="ExternalInput")
    b1 = nc.dram_tensor("b1", [F2], f32, kind="ExternalInput")
    w2 = nc.dram_tensor("w2", [F2, C], f32, kind="ExternalInput")
    b2 = nc.dram_tensor("b2", [C], f32, kind="ExternalInput")
    vec_out = nc.dram_tensor("vectors", [NSEG, F], f32, kind="ExternalOutput")
    conn_out = nc.dram_tensor("connections", [C, NSEG, NSEG], f32, kind="ExternalOutput")

    T = 4096
    ntiles = HW // T

    with tile.TileContext(nc) as tc:
        with (
            tc.tile_pool(name="xp", bufs=2) as xp,
            tc.tile_pool(name="idp", bufs=2) as idp,
            tc.tile_pool(name="mskd", bufs=2) as mp,
            tc.tile_pool(name="redp", bufs=2) as rp,
            tc.tile_pool(name="singles", bufs=1) as sp,
            tc.tile_pool(name="psum", bufs=1, space="PSUM") as pp,
        ):
            # ---- segment max over pixels (dense masked reduction) ----
            # Use all 128 partitions: p in [0,64) = features of pixel block A,
            # p in [64,128) = features of pixel block B (2 blocks of T pixels
            # per iteration), then fold the halves at the end.
            acc = sp.tile([128, NSEG + 1], f32)
            nc.vector.memset(acc, 0.0)

            for it in range(ntiles // 2):
                off = it * 2 * T
                x_t = xp.tile([128, T], f32)
                enc_pair = bass.AP(tensor=enc, offset=off,
                                   ap=[[T, 2], [HW, F], [1, T]])
                nc.gpsimd.dma_start(out=x_t, in_=enc_pair)
                ids_t = idp.tile([128, T], f32)
                ids_pair = bass.AP(tensor=msk, offset=off,
                                   ap=[[T, 2], [0, F], [1, T]])
                nc.gpsimd.dma_start(out=ids_t, in_=ids_pair)  # i32->f32 + bcast
                red = rp.tile([128, NSEG + 1], f32)
                for k in range(1, NSEG + 1):
                    m_t = mp.tile([128, T], f32)
                    nc.vector.scalar_tensor_tensor(
                        out=m_t, in0=ids_t, scalar=float(k), in1=x_t,
                        op0=Alu.is_equal, op1=Alu.mult,
                    )
                    nc.vector.tensor_reduce(
                        out=red[:, k:k + 1], in_=m_t, axis=mybir.AxisListType.X,
                        op=Alu.max,
                    )
                nc.vector.tensor_tensor(
                    out=acc[:, 1:], in0=acc[:, 1:], in1=red[:, 1:], op=Alu.max
                )

            # fold partition halves: vecs = max(acc[0:64], acc[64:128])
            accB = sp.tile([F, NSEG + 1], f32)
            nc.gpsimd.dma_start(out=accB, in_=acc[F:, :])
            vecs32 = sp.tile([F, NSEG + 1], f32)
            nc.vector.tensor_tensor(out=vecs32, in0=acc[0:F, :], in1=accB,
                                    op=Alu.max)
            vecs = vecs32[:, 1:]  # [F=64 partitions, NSEG=32] = v^T

            # write vectors output: [NSEG, F] = transpose of vecs
            nc.gpsimd.dma_start(out=vec_out[:, :].rearrange("n f -> f n"), in_=vecs)

            # ---- folded pairwise MLP ----
            # W = w1 @ w2 -> [2F, C]; A = W[:F], Bm = W[F:]; c0 = b1 @ w2 + b2
            w1T = sp.tile([F2, 2 * F], f32)
            nc.gpsimd.dma_start(out=w1T, in_=w1[:, :].rearrange("k h -> h k"))
            w2_sb = sp.tile([F2, C], f32)
            nc.gpsimd.dma_start(out=w2_sb, in_=w2[:, :])
            b1_sb = sp.tile([F2, 1], f32)
            nc.gpsimd.dma_start(out=b1_sb, in_=b1[:].rearrange("h -> h ()"))
            b2_sb = sp.tile([1, C], f32)
            nc.gpsimd.dma_start(out=b2_sb, in_=b2[:].rearrange("c -> () c"))

            A_ps = pp.tile([F, C], f32, tag="A_ps")
            nc.tensor.matmul(A_ps, lhsT=w1T[:, 0:F], rhs=w2_sb, start=True, stop=True)
            A_sb = sp.tile([F, C], f32)
            nc.vector.tensor_copy(out=A_sb, in_=A_ps)
            B_ps = pp.tile([F, C], f32, tag="B_ps")
            nc.tensor.matmul(B_ps, lhsT=w1T[:, F:], rhs=w2_sb, start=True, stop=True)
            B_sb = sp.tile([F, C], f32)
            nc.vector.tensor_copy(out=B_sb, in_=B_ps)

            c0_ps = pp.tile([1, C], f32)
            nc.tensor.matmul(c0_ps, lhsT=b1_sb, rhs=w2_sb, start=True, stop=True)
            c0_sb = sp.tile([1, C], f32)
            nc.vector.tensor_tensor(out=c0_sb, in0=c0_ps, in1=b2_sb, op=Alu.add)

            # P[i, c] = sum_f v[i, f] A[f, c]   (lhsT = vecs [F, NSEG])
            P_ps = pp.tile([NSEG, C], f32)
            nc.tensor.matmul(P_ps, lhsT=vecs, rhs=A_sb, start=True, stop=True)
            P_sb = sp.tile([NSEG, C], f32)
            nc.vector.tensor_copy(out=P_sb, in_=P_ps)
            # Qt_c[j] = sum_f B[f, c] v[j, f]  — one [1, NSEG] row per channel,
            # each at base partition 0 so it can feed the broadcast matmul.
            Qt_sb = sp.tile([1, C, NSEG], f32)
            for c in range(C):
                qt_ps = pp.tile([1, NSEG], f32, tag="qt_ps")
                nc.tensor.matmul(
                    qt_ps, lhsT=B_sb[:, c:c + 1], rhs=vecs, start=True, stop=True
                )
                nc.vector.tensor_copy(out=Qt_sb[:, c, :], in_=qt_ps)

            # broadcast helpers via ones-matmul
            ones_sb = sp.tile([1, NSEG], f32)
            nc.vector.memset(ones_sb, 1.0)
            c0b_ps = pp.tile([NSEG, C], f32)
            nc.tensor.matmul(c0b_ps, lhsT=ones_sb, rhs=c0_sb, start=True, stop=True)
            c0b_sb = sp.tile([NSEG, C], f32)
            nc.vector.tensor_copy(out=c0b_sb, in_=c0b_ps)

            conn_sb = sp.tile([NSEG, C, NSEG], f32)  # [i, c, j]
            for c in range(C):
                qb_ps = pp.tile([NSEG, NSEG], f32)
                nc.tensor.matmul(
                    qb_ps, lhsT=ones_sb, rhs=Qt_sb[:, c, :], start=True, stop=True
                )
                qb_sb = sp.tile([NSEG, NSEG], f32, tag="qb_sb")
                nc.vector.tensor_copy(out=qb_sb, in_=qb_ps)
                s_sb = sp.tile([NSEG, NSEG], f32, tag="s_sb")
                nc.vector.tensor_scalar(
                    out=s_sb, in0=qb_sb, scalar1=P_sb[:, c:c + 1], scalar2=None,
                    op0=Alu.add,
                )
                nc.scalar.activation(
                    out=conn_sb[:, c, :], in_=s_sb, func=Act.Sigmoid,
                    bias=c0b_sb[:, c:c + 1],
                )
            # connections[c, j, i] <- conn_sb[i, c, j]
            nc.gpsimd.dma_start(
                out=conn_out[:, :, :].rearrange("c j i -> i c j"), in_=conn_sb
            )

    nc.finalize()
    return nc


def _get_program():
    if "nc" not in _cached:
        _cached["nc"] = _build_program()
    return _cached["nc"]


def kernel(encoded, masks, w1, b1, w2, b2, _trace=False):
    from concourse.bass_utils import run_bass_kernel_spmd

    nc = _get_program()
    encoded = np.ascontiguousarray(encoded, dtype=np.float32)
    masks = np.ascontiguousarray(masks, dtype=np.int32)
    in_maps = []
    for b in range(B):
        in_maps.append({
            "encoded": encoded[b].reshape(F, HW),
            "masks": masks[b, 0].reshape(HW),
            "w1": np.ascontiguousarray(w1, dtype=np.float32),
            "b1": np.ascontiguousarray(b1, dtype=np.float32),
            "w2": np.ascontiguousarray(w2, dtype=np.float32),
            "b2": np.ascontiguousarray(b2, dtype=np.float32),
        })
    res = run_bass_kernel_spmd(nc, in_maps, core_ids=list(range(B)), trace=_trace)
    _cached["last_result"] = res
    vectors = np.stack([res.results[b]["vectors"] for b in range(B)])
    connections = np.stack([res.results[b]["connections"] for b in range(B)])
    return vectors, connections
